# revision 12
# baseline (speedup 1.0000x reference)
"""Llama attention (B=1, T=2048, D=2048, 32 q-heads, 8 kv-heads, hd=64) on 8 TRN2 cores.

Tensor-parallel over heads: core c owns q-heads 4c..4c+3 and kv-head c.
Each core computes its 4 heads' attention probabilities (written transposed,
[tk, tq]) plus its partial out = O @ Wo[rows]; host sums partials and
returns zero-copy transposed views for attn.
"""

import sys

if "/opt/trn_rl_repo" not in sys.path:
    sys.path.insert(0, "/opt/trn_rl_repo")

import numpy as np

T = 2048
D = 2048
N_HEAD = 32
N_KV = 8
HD = 64
NCORES = 8
HPC = N_HEAD // NCORES  # 4 q-heads per core
CH = 512                # tq chunk width
NCH = T // CH           # 4 chunks
NM = T // 128           # 16 tk tiles
KD = D // 128           # 16 contraction tiles for projections

_cache = {}


def _build():
    import concourse.bacc as bacc
    import concourse.tile as tile
    from concourse import mybir

    f32 = mybir.dt.float32
    Exp = mybir.ActivationFunctionType.Exp

    nc = bacc.Bacc("TRN2", target_bir_lowering=False)

    xT = nc.dram_tensor("xT", [D, T], f32, kind="ExternalInput")
    AT = nc.dram_tensor("AT", [T, T], f32, kind="ExternalInput")
    Wq = nc.dram_tensor("Wq", [D, HPC * HD], f32, kind="ExternalInput")
    Wkv = nc.dram_tensor("Wkv", [D, 2 * HD], f32, kind="ExternalInput")
    Wo = nc.dram_tensor("Wo", [HPC * HD, D], f32, kind="ExternalInput")
    CC = nc.dram_tensor("CC", [128, T], f32, kind="ExternalInput")
    SS2 = nc.dram_tensor("SS2", [128, T], f32, kind="ExternalInput")
    IDT_d = nc.dram_tensor("IDT", [128, 128], f32, kind="ExternalInput")

    attnT = nc.dram_tensor("attnT", [HPC, T, T], f32, kind="ExternalOutput")
    rsc = nc.dram_tensor("rsc", [NCH * HPC, 2, T], f32)
    outp = nc.dram_tensor("outp", [T, D], f32, kind="ExternalOutput")

    xT_r = xT.rearrange("(k p) t -> p k t", p=128)      # [128, 16, 2048]
    Wq_r = Wq.rearrange("(k p) c -> p k c", p=128)      # [128, 16, 256]
    Wkv_r = Wkv.rearrange("(k p) c -> p k c", p=128)    # [128, 16, 128]
    Wo_r = Wo.rearrange("(k p) e -> p k e", p=128)      # [128, 2, 2048]

    with tile.TileContext(nc) as tc:
        with tc.tile_pool(name="persist", bufs=1) as persist:
            qt = persist.tile([128, 2, T], f32, tag="qt")        # Q^T (2 head-pairs)
            kvt = persist.tile([128, T], f32, tag="kvt")         # 0:64 K^T, 64:128 V^T
            kt2 = persist.tile([128, T], f32, tag="kt2")         # K^T replicated 2x
            vaug_e = persist.tile([128, NM, HD + 1], f32, tag="vaug_e")  # [V | 1]
            osb = persist.tile([128, 2, T], f32, tag="osb")      # O^T [256, 2048]
            wo_sb = persist.tile([128, 2, D], f32, tag="wo")
            idt = persist.tile([128, 128], f32, tag="idt")
            ones1 = persist.tile([1, 128], f32, tag="ones1")

            nc.sync.dma_start(idt[:, :], IDT_d[:, :])
            nc.vector.memset(ones1[:, :], 1.0)
            nc.sync.dma_start(wo_sb[:, :, :], Wo_r)

            # ---- Phase 1a: projections (xT-resident) ----
            with tc.tile_pool(name="ph1", bufs=1) as ph1, \
                 tc.tile_pool(name="ph1ps", bufs=2, space="PSUM") as ph1ps:
                wq_sb = ph1.tile([128, KD, HPC * HD], f32, tag="wq")
                wkv_sb = ph1.tile([128, KD, 2 * HD], f32, tag="wkv")
                nc.sync.dma_start(wq_sb[:, :, :], Wq_r)
                nc.sync.dma_start(wkv_sb[:, :, :], Wkv_r)

                KQ = 4  # k-tiles per xT quarter
                for q in range(KD // KQ):
                    xq = ph1.tile([128, KQ, T], f32, tag="xq", bufs=2)
                    nc.sync.dma_start(xq[:, :, :], xT_r[:, q * KQ:(q + 1) * KQ, :])
                    for mc in range(2):
                        for nt in range(4):
                            ps = ph1ps.tile([128, 512], f32, tag="pp")
                            for kk in range(KQ):
                                nc.tensor.matmul(
                                    ps[:, :],
                                    lhsT=wq_sb[:, q * KQ + kk, mc * 128:(mc + 1) * 128],
                                    rhs=xq[:, kk, nt * 512:(nt + 1) * 512],
                                    start=(kk == 0), stop=(kk == KQ - 1),
                                )
                            dst = qt[:, mc, nt * 512:(nt + 1) * 512]
                            if q == 0:
                                nc.scalar.copy(dst, ps[:, :])
                            else:
                                nc.vector.tensor_add(dst, dst, ps[:, :])
                    for nt in range(4):
                        ps = ph1ps.tile([128, 512], f32, tag="pp")
                        for kk in range(KQ):
                            nc.tensor.matmul(
                                ps[:, :],
                                lhsT=wkv_sb[:, q * KQ + kk, :],
                                rhs=xq[:, kk, nt * 512:(nt + 1) * 512],
                                start=(kk == 0), stop=(kk == KQ - 1),
                            )
                        dst = kvt[:, nt * 512:(nt + 1) * 512]
                        if q == 0:
                            nc.scalar.copy(dst, ps[:, :])
                        else:
                            nc.vector.tensor_add(dst, dst, ps[:, :])

            # ---- Phase 1b: RoPE (Q, K) + K replicate + V transpose/augment ----
            with tc.tile_pool(name="ph1b", bufs=2) as ph1b, \
                 tc.tile_pool(name="ph1bps", bufs=2, space="PSUM") as ph1bps:
                cc = ph1b.tile([128, T], f32, tag="cc")
                ss2 = ph1b.tile([128, T], f32, tag="ss2")
                nc.sync.dma_start(cc[:, :], CC[:, :])
                nc.sync.dma_start(ss2[:, :], SS2[:, :])

                for mc in range(2):
                    sw = ph1b.tile([128, T], f32, tag="sw")
                    for b in range(2):  # swap 32-row e/o blocks per head
                        nc.sync.dma_start(sw[b * 64:b * 64 + 32, :],
                                          qt[b * 64 + 32:b * 64 + 64, mc, :])
                        nc.sync.dma_start(sw[b * 64 + 32:b * 64 + 64, :],
                                          qt[b * 64:b * 64 + 32, mc, :])
                    nc.vector.tensor_mul(qt[:, mc, :], qt[:, mc, :], cc[:, :])
                    nc.vector.tensor_mul(sw[:, :], sw[:, :], ss2[:, :])
                    nc.vector.tensor_add(qt[:, mc, :], qt[:, mc, :], sw[:, :])
                ksw = ph1b.tile([64, T], f32, tag="sw")
                nc.sync.dma_start(ksw[0:32, :], kvt[32:64, :])
                nc.sync.dma_start(ksw[32:64, :], kvt[0:32, :])
                nc.vector.tensor_mul(kvt[0:64, :], kvt[0:64, :], cc[0:64, :])
                nc.vector.tensor_mul(ksw[0:32, :], ksw[0:32, :], ss2[0:32, :])
                nc.vector.tensor_mul(ksw[32:64, :], ksw[32:64, :], ss2[32:64, :])
                nc.vector.tensor_add(kvt[0:64, :], kvt[0:64, :], ksw[0:64, :])
                nc.sync.dma_start(kt2[0:64, :], kvt[0:64, :])
                nc.sync.dma_start(kt2[64:128, :], kvt[0:64, :])

                nc.vector.memset(vaug_e[:, :, HD:HD + 1], 1.0)
                for m in range(NM):
                    pv = ph1bps.tile([128, 64], f32, tag="pv")
                    nc.tensor.transpose(pv[:, :], kvt[64:128, m * 128:(m + 1) * 128],
                                        idt[64:128, 64:128])
                    nc.scalar.copy(vaug_e[:, m, 0:HD], pv[:, :])

            # ---- Phase 2: per-chunk attention, heads in (even, odd) pairs ----
            with tc.tile_pool(name="esbp", bufs=1) as esbp, \
                 tc.tile_pool(name="atp", bufs=1) as atp, \
                 tc.tile_pool(name="rp", bufs=2) as rp, \
                 tc.tile_pool(name="sps", bufs=2, space="PSUM") as sps, \
                 tc.tile_pool(name="ops", bufs=1, space="PSUM") as ops, \
                 tc.tile_pool(name="wps", bufs=1, space="PSUM") as wps, \
                 tc.tile_pool(name="wcp", bufs=2) as wcp:
                for ch in range(NCH):
                    cs = ch * CH
                    at_all = atp.tile([128, NM, CH], f32, tag="at")
                    nc.sync.dma_start(
                        at_all[:, :, :],
                        AT[:, cs:cs + CH].rearrange("(m p) q -> p m q", p=128))
                    for par in range(2):        # 0: heads 0,2  1: heads 1,3
                        pb = par * 64           # partition base
                        heads = (0 + par, 2 + par)
                        vaug = vaug_e
                        obase, sumrow = 0, 64
                        po = {h: ops.tile([128, CH], f32, tag=f"po{h}", name=f"po{h}")
                              for h in heads}
                        esbs = {h: esbp.tile([128, NM, CH], f32, tag=f"esb{h // 2}", name=f"esb{h // 2}")
                                for h in heads}
                        for m in range(NM):
                            for h in heads:
                                pss = sps.tile([128, CH], f32, tag="ps")
                                nc.tensor.matmul(
                                    pss[:, :],
                                    lhsT=kt2[pb:pb + 64, m * 128:(m + 1) * 128],
                                    rhs=qt[pb:pb + 64, h // 2, cs:cs + CH],
                                    start=True, stop=False,
                                )
                                nc.tensor.matmul(
                                    pss[:, :], lhsT=idt[:, :], rhs=at_all[:, m, :],
                                    start=False, stop=True, skip_group_check=True,
                                )
                                nc.scalar.activation(esbs[h][:, m, :], pss[:, :], Exp)
                                nc.tensor.matmul(
                                    po[h][obase:obase + 65, :],
                                    lhsT=vaug[:, m, :], rhs=esbs[h][:, m, :],
                                    start=(m == 0), stop=(m == NM - 1),
                                )
                        for h in heads:
                            esb = esbs[h]
                            rsb = rp.tile([1, CH], f32, tag="rsb")
                            rpk = rp.tile([32, CH // 32], f32, tag="rpk")
                            rb = rp.tile([128, CH], f32, tag="rb")
                            idx = ch * HPC + h
                            nc.scalar.copy(rsb[0:1, :], po[h][sumrow:sumrow + 1, :])
                            nc.sync.dma_start(rsc[idx, 0, 0:CH], rsb[0:1, :])
                            nc.sync.dma_start(
                                rpk[:, :],
                                rsc[idx, 0, 0:CH].rearrange("(p f) -> p f", p=32))
                            nc.vector.reciprocal(rpk[:, :], rpk[:, :])
                            nc.sync.dma_start(
                                rsc[idx, 1, 0:CH].rearrange("(p f) -> p f", p=32),
                                rpk[:, :])
                            nc.sync.dma_start(rsb[0:1, :], rsc[idx, 1, 0:CH])
                            rb_ps = sps.tile([128, CH], f32, tag="ps", name="rb_ps")
                            nc.tensor.matmul(rb_ps[:, :], lhsT=ones1[0:1, :],
                                             rhs=rsb[0:1, :], start=True, stop=True)
                            nc.scalar.copy(rb[:, :], rb_ps[:, :])
                            nc.vector.tensor_mul(
                                esb[:, :, :], esb[:, :, :],
                                rb[:, :].rearrange("p (a f) -> p a f", a=1)
                                    .to_broadcast((128, NM, CH)),
                            )
                            if par == 0:
                                nc.vector.tensor_mul(
                                    osb[0:64, h // 2, cs:cs + CH],
                                    po[h][0:64, :], rb[0:64, :])
                            else:
                                ost = rp.tile([64, CH], f32, tag="ost")
                                nc.vector.tensor_mul(
                                    ost[:, :], po[h][0:64, :], rb[0:64, :])
                                nc.sync.dma_start(
                                    osb[64:128, h // 2, cs:cs + CH], ost[:, :])
                            nc.sync.dma_start(
                                attnT[h, :, cs:cs + CH].rearrange(
                                    "(m p) q -> p m q", p=128),
                                esb[:, :, :],
                            )

                # ---- Phase 3: partial out = O @ Wo_rows ----
                for mt in range(NM):
                    wt = wcp.tile([128, D], f32, tag="wt")
                    for nt in range(4):
                        pw = wps.tile([128, 512], f32, tag="pw")
                        for k in range(2):
                            nc.tensor.matmul(
                                pw[:, :],
                                lhsT=osb[:, k, mt * 128:(mt + 1) * 128],
                                rhs=wo_sb[:, k, nt * 512:(nt + 1) * 512],
                                start=(k == 0), stop=(k == 1),
                            )
                        nc.vector.tensor_copy(wt[:, nt * 512:(nt + 1) * 512], pw[:, :])
                    nc.sync.dma_start(outp[mt * 128:(mt + 1) * 128, :], wt[:, :])

    nc.compile()
    return nc


def _host_inputs(x, Wq, Wk, Wv, Wo, mask):
    x2 = np.ascontiguousarray(x.reshape(T, D), dtype=np.float32)
    xT = np.ascontiguousarray(x2.T)
    AT = np.where(mask.reshape(T, T).T == 0, np.float32(-1e9), np.float32(0.0))
    AT = np.ascontiguousarray(AT, dtype=np.float32)

    # even/odd channel permutation within each head (interleaved -> block rope)
    perm = np.concatenate([np.arange(0, HD, 2), np.arange(1, HD, 2)])

    i = np.arange(32, dtype=np.float64)
    theta = 10000.0 ** (-2.0 * i / HD)
    t = np.arange(T, dtype=np.float64)
    ang = theta[:, None] * t[None, :]          # [32, T]
    cosb = np.cos(ang).astype(np.float32)
    sinb = np.sin(ang).astype(np.float32)
    CCh = np.ascontiguousarray(np.concatenate([cosb, cosb, cosb, cosb], axis=0))
    SS2h = np.ascontiguousarray(np.concatenate([-sinb, sinb, -sinb, sinb], axis=0))
    IDT = np.eye(128, dtype=np.float32)

    in_maps = []
    for c in range(NCORES):
        wq_c = Wq[:, c * HPC * HD:(c + 1) * HPC * HD].astype(np.float32) * np.float32(0.25)
        wq_p = np.empty_like(wq_c)
        for h in range(HPC):
            wq_p[:, h * HD:(h + 1) * HD] = wq_c[:, h * HD + perm]
        wk_c = Wk[:, c * HD:(c + 1) * HD][:, perm]
        wv_c = Wv[:, c * HD:(c + 1) * HD]
        wkv = np.ascontiguousarray(
            np.concatenate([wk_c, wv_c], axis=1), dtype=np.float32)
        wo_c = np.ascontiguousarray(
            Wo[c * HPC * HD:(c + 1) * HPC * HD, :], dtype=np.float32)
        in_maps.append({
            "xT": xT, "AT": AT,
            "Wq": np.ascontiguousarray(wq_p),
            "Wkv": wkv, "Wo": wo_c,
            "CC": CCh, "SS2": SS2h, "IDT": IDT,
        })
    return in_maps


def _run(in_maps, trace=False):
    from concourse.bass_utils import run_bass_kernel_spmd
    if "nc" not in _cache:
        _cache["nc"] = _build()
    return run_bass_kernel_spmd(
        _cache["nc"], in_maps, core_ids=list(range(NCORES)), trace=trace)


def kernel(x, Wq, Wk, Wv, Wo, mask, _trace=False):
    in_maps = _host_inputs(
        np.asarray(x), np.asarray(Wq), np.asarray(Wk),
        np.asarray(Wv), np.asarray(Wo), np.asarray(mask))
    res = _run(in_maps, trace=_trace)
    outs = res.results
    _cache["last_results"] = res

    out = outs[0]["outp"].astype(np.float32).copy()
    for c in range(1, NCORES):
        out += outs[c]["outp"]
    attnT_all = np.stack([outs[c]["attnT"] for c in range(NCORES)])  # [8,4,T,T]
    attn = attnT_all.reshape(N_HEAD, T, T).transpose(0, 2, 1)[None]  # view
    return out.reshape(1, T, D), attn
